# revision 28
# baseline (speedup 1.0000x reference)
"""Multi-head attention forward kernel for Trainium2 (8 NeuronCores).

Problem: B=2, N=2048, C=1024, H=16 heads, head_dim=64.
    q = x @ Wq.T + bq  (same for k, v)
    out = softmax(q k^T / sqrt(C)) v       (per head), re-merged to [B, N, C]

Sharding: core = (batch b, head-group g): b = core // 4, g = core % 4.
Each core computes 4 heads of one batch element. No collectives needed --
outputs are disjoint; host gathers and finishes with a cheap epilogue
(normalize by the row-sums and transpose).

v4 design (trace-driven evolution of the 188us baseline):
  - The EXP stream on ACT (128 ops x ~1.05us) is the hard floor; the
    kernel is one flat 128-window pipeline (window = one key chunk of one
    (pair, query-block)): QK pair -> EXP -> static filler -> lagged PV.
  - PV/ssum trail the exp stream by L=6 windows and cross query-block
    boundaries (two O^T PSUM accumulators in flight), so the PE overflow
    of the v-projection-heavy first windows spreads into pair-1's slack
    instead of starving ACT.
  - Startup: weights arrive pair-split and pre-packed in SBUF layout
    (one contiguous 256KB DMA each); x columns stream in 256/512-col
    slabs.  The first q/k projections chase the per-chunk DMAs, so the
    first EXP fires at ~13us instead of ~29us.  DMA issue order is the
    priority order (descriptors of each dma_start spread over all 16
    queues; the phase runs at aggregate HBM bandwidth).
  - kt0 blocks are computed in N=256 column sub-blocks chasing the x
    slabs; remaining projection blocks are spread as small parts over
    windows with PE slack (pair-1 carries the late qt1 blocks only).
  - Softmax denominators: DVE folds the two fp16 parity accumulators,
    then one ones-matmul per head (PSUM partitions {0,32} via
    tile_position) -- half the baseline's reduction matmuls.
  - PSUM budget (8 banks): st double-buffer 4 + two O^T accumulators 2 +
    shared proj/ones pool 2.
Outputs: out_o [2, 128, N] bf16 (pair, head-major O^T rows, queries),
         out_s [2, 2, N]   f32 (pair, head, query sums).
"""

import os
import sys

import ml_dtypes
import numpy as np

for _p in ("/opt/trn_rl_repo",):
    if _p not in sys.path:
        sys.path.insert(0, _p)

import concourse.bass as bass  # noqa: E402
import concourse.tile as tile  # noqa: E402
from concourse import bacc, mybir  # noqa: E402
from concourse.bass_utils import run_bass_kernel_spmd  # noqa: E402

N = 2048  # sequence length
C = 1024  # model dim
D = 64  # head dim
NH = 4  # heads per core
HD = NH * D  # 256 output channels per core
NCORES = 8
KB = N // 128  # 16 key chunks of 128
QB = N // 512  # 4 query blocks of 512
KC = C // 128  # 8 contraction chunks for projections
SCALE = 1.0 / 32.0  # 1 / sqrt(C)
LAG = 6  # PV/ssum windows behind the exp stream

F32 = mybir.dt.float32
BF16 = mybir.dt.bfloat16
FP16 = mybir.dt.float16


def build_kernel(tc, xt, wq_d, wk_d, wv_d, bq, bk, bv, out_o, out_s):
    nc = tc.nc
    Exp = mybir.ActivationFunctionType.Exp

    with (
        tc.tile_pool(name="res", bufs=1) as res,
        tc.tile_pool(name="ppsum", bufs=2, space="PSUM") as ppsum,
        tc.tile_pool(name="stp", bufs=2, space="PSUM") as stp,
        tc.tile_pool(name="opp", bufs=2, space="PSUM") as opp,
        tc.tile_pool(name="ptp", bufs=23) as ptp,
        tc.tile_pool(name="otp", bufs=2) as otp,
        tc.tile_pool(name="ssp", bufs=2) as ssp,
    ):
        # ---- resident SBUF tensors ----
        # weights arrive pre-packed per head-pair: [128, KC, 128]
        wq_p = [res.tile([128, KC, 128], BF16, tag=f"wq{m}", name=f"wq{m}") for m in range(2)]
        wk_p = [res.tile([128, KC, 128], BF16, tag=f"wk{m}", name=f"wk{m}") for m in range(2)]
        wv_all = res.tile([128, KC, HD], BF16, tag="wv", name="wv")
        xt_sb = [res.tile([128, N], BF16, tag=f"xt{k}", name=f"xt{k}") for k in range(KC)]
        wv_sb = [wv_all[:, k, :] for k in range(KC)]
        qt_sb = [res.tile([128, N], BF16, tag=f"qt{m}", name=f"qt{m}") for m in range(2)]
        kt_sb = [res.tile([128, N], BF16, tag=f"kt{m}", name=f"kt{m}") for m in range(2)]
        v_sb = [res.tile([128, NH, D], FP16, tag=f"v{kb}", name=f"v{kb}") for kb in range(KB)]
        bq_sb = [res.tile([128, 1], F32, tag=f"bq{m}", name=f"bq{m}") for m in range(2)]
        bk_sb = [res.tile([128, 1], F32, tag=f"bk{m}", name=f"bk{m}") for m in range(2)]
        bv_sb = res.tile([128, HD], F32, tag="bv", name="bv")
        ones_sb = res.tile([128, 1], FP16, tag="ones", name="ones")
        warm_sb = res.tile([1, 2], F32, tag="warm", name="warm")
        warmmm_sb = res.tile([128, 64], BF16, tag="warmmm", name="warmmm")

        # ---- input DMAs in strict priority order ----
        nc.sync.dma_start(out=wq_p[0][:], in_=wq_d[0])
        nc.sync.dma_start(out=wk_p[0][:], in_=wk_d[0])
        for m in range(2):
            sl = slice(m * 128, (m + 1) * 128)
            nc.sync.dma_start(out=bq_sb[m][:], in_=bq[sl])
            nc.sync.dma_start(out=bk_sb[m][:], in_=bk[sl])
        bv_bcast = bass.AP(tensor=bv.tensor, offset=bv.offset, ap=[[0, 128]] + list(bv.ap))
        nc.sync.dma_start(out=bv_sb[:], in_=bv_bcast)
        for k in range(KC):
            nc.sync.dma_start(out=xt_sb[k][:, 0:512], in_=xt[k * 128 : (k + 1) * 128, 0:512])
        nc.sync.dma_start(out=wv_all[:], in_=wv_d.rearrange("(k p) n -> p k n", p=128))
        for j in range(2, 8):  # x columns 512:2048 in 256-col slabs
            for k in range(KC):
                nc.sync.dma_start(
                    out=xt_sb[k][:, j * 256 : (j + 1) * 256],
                    in_=xt[k * 128 : (k + 1) * 128, j * 256 : (j + 1) * 256],
                )
        nc.sync.dma_start(out=wq_p[1][:], in_=wq_d[1])
        nc.sync.dma_start(out=wk_p[1][:], in_=wk_d[1])

        nc.vector.memset(ones_sb[:], 1.0)
        # warm up the ACT exp table while DMAs land
        nc.vector.memset(warm_sb[:], 0.0)
        nc.scalar.activation(out=warm_sb[:, 0:1], in_=warm_sb[:, 1:2], func=Exp)
        # warm up the PE (HAM un-throttles after ~3.4us of sustained
        # activity) on junk data so the prologue projections run at 2.4GHz
        nc.vector.memset(warmmm_sb[:], 0.5)
        wps = ppsum.tile([64, 64], F32, tag="qkps", name="wps")
        for i in range(32):
            nc.tensor.matmul(
                out=wps[:],
                lhsT=warmmm_sb[:, 0:64],
                rhs=warmmm_sb[:],
                start=(i == 0),
                stop=(i == 31),
            )

        # ---- building blocks ----
        def proj_qk_part(state, which, m, nb, k0, k1):
            """Chunks [k0, k1) of a q/k projection block [128, 512]."""
            w_p = (wq_p if which == "q" else wk_p)[m]
            nsl = slice(nb * 512, (nb + 1) * 512)
            if k0 == 0:
                state["ps"] = ppsum.tile([128, 512], F32, tag="qkps", name="qkps")
            ps = state["ps"]
            for k in range(k0, k1):
                nc.tensor.matmul(
                    out=ps[:],
                    lhsT=w_p[:, k, :],
                    rhs=xt_sb[k][:, nsl],
                    start=(k == 0),
                    stop=(k == KC - 1),
                )
            if k1 == KC:
                b_sb = (bq_sb if which == "q" else bk_sb)[m]
                t_sb = (qt_sb if which == "q" else kt_sb)[m]
                nc.vector.tensor_scalar_add(out=t_sb[:, nsl], in0=ps[:], scalar1=b_sb[:])

        def proj_qk_256(which, m, nb2):
            """One N=256 column sub-block of a q/k projection (chases the
            256-col x slabs)."""
            w_p = (wq_p if which == "q" else wk_p)[m]
            nsl = slice(nb2 * 256, (nb2 + 1) * 256)
            ps = ppsum.tile([128, 256], F32, tag="qkps", name="qkps2")
            for k in range(KC):
                nc.tensor.matmul(
                    out=ps[:],
                    lhsT=w_p[:, k, :],
                    rhs=xt_sb[k][:, nsl],
                    start=(k == 0),
                    stop=(k == KC - 1),
                )
            b_sb = (bq_sb if which == "q" else bk_sb)[m]
            t_sb = (qt_sb if which == "q" else kt_sb)[m]
            nc.vector.tensor_scalar_add(out=t_sb[:, nsl], in0=ps[:], scalar1=b_sb[:])

        def proj_v_block(kb):
            vps = ppsum.tile([128, HD], F32, tag="qkps", name="vps")
            for k in range(KC):
                nc.tensor.matmul(
                    out=vps[:],
                    lhsT=xt_sb[k][:, kb * 128 : (kb + 1) * 128],
                    rhs=wv_sb[k][:],
                    start=(k == 0),
                    stop=(k == KC - 1),
                )
            nc.vector.tensor_add(
                out=v_sb[kb][:],
                in0=vps[:].rearrange("p (h d) -> p h d", h=NH),
                in1=bv_sb[:].rearrange("p (h d) -> p h d", h=NH),
            )

        # ---- static filler schedule: window -> list of closures ----
        filler = {}

        def sched(w, fn):
            filler.setdefault(w, []).append(fn)

        def sched_parts(windows, which, m, nb):
            st = {}
            bounds = [round(i * KC / len(windows)) for i in range(len(windows) + 1)]
            for w, k0, k1 in zip(windows, bounds, bounds[1:]):
                sched(w, lambda st=st, k0=k0, k1=k1: proj_qk_part(st, which, m, nb, k0, k1))

        # v blocks at half density over the first ~32 windows (the first
        # query block's PV trails the exp stream by LAG0=20 windows, so
        # v(kb) is only needed by window kb+20)
        V_SCHED = (3, 4, 6, 8, 10, 12, 14, 16, 18, 20, 22, 24, 26, 28, 30, 31)
        for kb in range(KB):
            sched(V_SCHED[kb], lambda kb=kb: proj_v_block(kb))
        # kt0 column sub-blocks chase the x slab DMAs
        for w, nb2 in ((1, 2), (2, 3), (5, 4), (7, 5), (9, 6), (11, 7)):
            sched(w, lambda nb2=nb2: proj_qk_256("k", 0, nb2))
        sched_parts([12, 14], "q", 0, 1)
        sched_parts([17, 19, 21, 23], "q", 0, 2)
        sched_parts([25, 27, 29, 33], "q", 0, 3)
        sched_parts([34, 35, 36, 37], "k", 1, 0)
        sched_parts([38, 39, 40, 41], "k", 1, 1)
        sched_parts([43, 44, 45, 46], "k", 1, 2)
        sched_parts([48, 49, 50, 51], "k", 1, 3)
        sched_parts([53, 54, 55, 56], "q", 1, 0)
        sched_parts([72, 73, 74, 75], "q", 1, 1)
        sched_parts([88, 89, 90, 91], "q", 1, 2)
        sched_parts([104, 105, 106, 107], "q", 1, 3)

        # ---- prologue: first projections chase the per-chunk x DMAs ----
        k0ps = ppsum.tile([128, 512], F32, tag="qkps", name="k0ps")
        q0ps = ppsum.tile([128, 512], F32, tag="qkps", name="q0ps")
        for k in range(KC):
            nc.tensor.matmul(
                out=k0ps[:], lhsT=wk_p[0][:, k, :], rhs=xt_sb[k][:, 0:512],
                start=(k == 0), stop=(k == KC - 1),
            )
            nc.tensor.matmul(
                out=q0ps[:], lhsT=wq_p[0][:, k, :], rhs=xt_sb[k][:, 0:512],
                start=(k == 0), stop=(k == KC - 1),
            )
        nc.vector.tensor_scalar_add(out=kt_sb[0][:, 0:512], in0=k0ps[:], scalar1=bk_sb[0][:])
        nc.vector.tensor_scalar_add(out=qt_sb[0][:, 0:512], in0=q0ps[:], scalar1=bq_sb[0][:])

        # ---- the flat lagged window pipeline ----
        qstate = {}

        def emit_pv(p, qb, kb, pt):
            if (p, qb) not in qstate:
                qstate[(p, qb)] = {
                    "o": opp.tile([128, 512], F32, tag="o", name="o2"),
                    "ssum": [
                        ssp.tile([128, 2, 512], FP16, tag=f"ssum{j}", name=f"ssum{j}")
                        for j in range(2)
                    ],
                }
            s = qstate[(p, qb)]
            o_ps = s["o"]
            for h in range(2):
                nc.tensor.matmul(
                    out=o_ps[h * D : (h + 1) * D, :],
                    lhsT=v_sb[kb][:, 2 * p + h, :],
                    rhs=pt[:, h, :],
                    start=(kb == 0),
                    stop=(kb == KB - 1),
                    tile_position=(0, h * D),
                    skip_group_check=True,
                )
            sj = s["ssum"][kb % 2]
            if kb < 2:
                nc.vector.tensor_copy(out=sj[:], in_=pt[:])
            else:
                nc.vector.tensor_add(out=sj[:], in0=sj[:], in1=pt[:])

        def epilogue(p, qb):
            s = qstate.pop((p, qb))
            qsl = slice(qb * 512, (qb + 1) * 512)
            ssum = s["ssum"]
            nc.vector.tensor_add(out=ssum[0][:], in0=ssum[0][:], in1=ssum[1][:])
            s_ps = ppsum.tile([33, 512], F32, tag="qkps", name="sps")
            for h in range(2):
                nc.tensor.matmul(
                    out=s_ps[32 * h : 32 * h + 1, :],
                    lhsT=ones_sb[:],
                    rhs=ssum[0][:, h, :],
                    start=True,
                    stop=True,
                    tile_position=(0, 32 * h),
                    skip_group_check=True,
                )
            ss = otp.tile([33, 512], F32, tag="ss", name="ss")
            for h in range(2):
                nc.vector.tensor_copy(
                    out=ss[32 * h : 32 * h + 1, :], in_=s_ps[32 * h : 32 * h + 1, :]
                )
            ss_view = bass.AP(
                tensor=ss.tensor, offset=ss.offset,
                ap=[[32 * ss.ap[0][0], 2]] + list(ss.ap[1:]),
            )
            nc.sync.dma_start(out=out_s[p, :, qsl], in_=ss_view)
            ot = otp.tile([128, 512], BF16, tag="ot", name="ot")
            nc.vector.tensor_copy(out=ot[:], in_=s["o"][:])
            nc.sync.dma_start(out=out_o[p, :, qsl], in_=ot[:])

        # Windows are processed in groups of two -- both QK pairs, then
        # both EXPs, then filler, then both lagged PVs -- so same-PE-mode
        # matmuls sit back to back and tiling-mode-switch drains happen
        # once per group instead of once per window.  The 2-slot st pool
        # still pipelines: QK(w+1) fills slot B while exp(w) reads A, and
        # QK(w+2) reuses A which exp(w) freed a full window earlier.
        windows = [(p, qb, kb) for p in range(2) for qb in range(QB) for kb in range(KB)]
        pending = {}

        def emit_qk(w):
            p, qb, kb = windows[w]
            qsl = slice(qb * 512, (qb + 1) * 512)
            ksl = slice(kb * 128, (kb + 1) * 128)
            st = stp.tile([128, 2, 512], F32, tag="st", name="st")
            for h in range(2):
                hsl = slice(h * D, (h + 1) * D)
                nc.tensor.matmul(
                    out=st[:, h, :],
                    lhsT=kt_sb[p][hsl, ksl],
                    rhs=qt_sb[p][hsl, qsl],
                    start=True,
                    stop=True,
                )
            return st

        def emit_exp(w, st):
            p, qb, kb = windows[w]
            pt = ptp.tile([128, 2, 512], FP16, tag="pt", name="pt")
            nc.scalar.activation(out=pt[:], in_=st[:], func=Exp, scale=SCALE)
            pending[w] = (p, qb, kb, pt)

        def drain_pv(wl):
            pl, ql, kl, ptl = pending.pop(wl)
            emit_pv(pl, ql, kl, ptl)
            if kl == KB - 1:
                epilogue(pl, ql)

        # Per-window drain time: the first query block trails by 20 windows
        # (v-block slack), the last few by only 2 (shorter serial tail);
        # everything else by LAG.  PSUM accumulation order within a qb may
        # interleave across qbs -- each qb has its own accumulator/chain.
        drains = {}
        for w in range(len(windows)):
            dw = w + (20 if w < 16 else (2 if w >= 120 else LAG))
            drains.setdefault(dw, []).append(w)
        for w in range(0, len(windows) + 24, 2):
            if w < len(windows):
                st_a = emit_qk(w)
                st_b = emit_qk(w + 1)
                emit_exp(w, st_a)
                emit_exp(w + 1, st_b)
                for fn in filler.get(w, ()):
                    fn()
                for fn in filler.get(w + 1, ()):
                    fn()
            for dw in (w, w + 1):
                for wl in drains.get(dw, ()):
                    drain_pv(wl)


def build_nc():
    nc = bacc.Bacc(
        "TRN2",
        target_bir_lowering=False,
        debug=False,
        num_devices=NCORES,
        enable_partition_id=False,
    )
    xt = nc.dram_tensor("xt", [C, N], BF16, kind="ExternalInput").ap()
    wq_d = [nc.dram_tensor(f"wq{m}", [128, KC, 128], BF16, kind="ExternalInput").ap() for m in range(2)]
    wk_d = [nc.dram_tensor(f"wk{m}", [128, KC, 128], BF16, kind="ExternalInput").ap() for m in range(2)]
    wv_d = nc.dram_tensor("wvt", [C, HD], BF16, kind="ExternalInput").ap()
    bq = nc.dram_tensor("bq", [HD], F32, kind="ExternalInput").ap()
    bk = nc.dram_tensor("bk", [HD], F32, kind="ExternalInput").ap()
    bv = nc.dram_tensor("bv", [HD], F32, kind="ExternalInput").ap()
    out_o = nc.dram_tensor("out_o", [2, 128, N], BF16, kind="ExternalOutput").ap()
    out_s = nc.dram_tensor("out_s", [2, 2, N], F32, kind="ExternalOutput").ap()

    with tile.TileContext(nc) as tc:
        build_kernel(tc, xt, wq_d, wk_d, wv_d, bq, bk, bv, out_o, out_s)
    nc.compile()
    return nc


def _pack_w(w, m):
    """[C, HD] transposed weight -> pair-m packed [128, KC, 128] bf16."""
    wt = np.asarray(w, np.float32)[:, m * 128 : (m + 1) * 128]  # [C, 128]
    return np.ascontiguousarray(
        wt.reshape(KC, 128, 128).transpose(1, 0, 2)
    ).astype(ml_dtypes.bfloat16)


def shard_inputs(inputs):
    x = np.asarray(inputs["x"], np.float32)
    in_maps = []
    for core in range(NCORES):
        b, g = core // 4, core % 4
        sl = slice(g * HD, (g + 1) * HD)
        wqt = np.asarray(inputs["Wq"], np.float32)[sl, :].T  # [C, HD]
        wkt = np.asarray(inputs["Wk"], np.float32)[sl, :].T
        wvt = np.asarray(inputs["Wv"], np.float32)[sl, :].T
        in_maps.append(
            {
                "xt": np.ascontiguousarray(x[b].T).astype(ml_dtypes.bfloat16),
                "wq0": _pack_w(wqt, 0),
                "wq1": _pack_w(wqt, 1),
                "wk0": _pack_w(wkt, 0),
                "wk1": _pack_w(wkt, 1),
                "wvt": np.ascontiguousarray(wvt).astype(ml_dtypes.bfloat16),
                "bq": np.ascontiguousarray(np.asarray(inputs["bq"], np.float32)[sl]),
                "bk": np.ascontiguousarray(np.asarray(inputs["bk"], np.float32)[sl]),
                "bv": np.ascontiguousarray(np.asarray(inputs["bv"], np.float32)[sl]),
            }
        )
    return in_maps


def assemble(results, B=2):
    out = np.zeros((B, N, C), np.float32)
    for core in range(NCORES):
        b, g = core // 4, core % 4
        oo = np.asarray(results[core]["out_o"], np.float32)  # [2, 128, N]
        os_ = np.asarray(results[core]["out_s"], np.float32)  # [2, 2, N]
        o = oo.reshape(2, 2, D, N)  # [pair, head, d, n]
        on = o / os_[:, :, None, :]
        # [pair, head, d, n] -> [n, pair*2*D + head*D + d]
        out[b, :, g * HD : (g + 1) * HD] = (
            on.transpose(3, 0, 1, 2).reshape(N, HD)
        )
    return out


_NC_CACHE = None


def _get_nc():
    global _NC_CACHE
    if _NC_CACHE is None:
        _NC_CACHE = build_nc()
    return _NC_CACHE


def kernel(**inputs):
    nc = _get_nc()
    in_maps = shard_inputs(inputs)
    res = run_bass_kernel_spmd(
        nc,
        in_maps,
        core_ids=list(range(NCORES)),
        trace=bool(int(os.environ.get("KERNEL_TRACE", "0"))),
    )
    return assemble(res.results, B=int(np.asarray(inputs["x"]).shape[0]))


# revision 33
# speedup vs baseline: 1.0457x; 1.0457x over previous
"""Multi-head attention forward kernel for Trainium2 (8 NeuronCores).

Problem: B=2, N=2048, C=1024, H=16 heads, head_dim=64.
    q = x @ Wq.T + bq  (same for k, v)
    out = softmax(q k^T / sqrt(C)) v       (per head), re-merged to [B, N, C]

Sharding: core = (batch b, head-group g): b = core // 4, g = core % 4.
Each core computes 4 heads of one batch element. No collectives needed --
outputs are disjoint; host gathers and finishes with a cheap epilogue
(normalize by the row-sums and transpose).

Design (trace-driven evolution of the 188us baseline):
  - The EXP stream on ACT (128 ops x ~1.05us, 1 elem/cycle/lane, the only
    engine with exp) is the hard floor; the kernel is one flat 128-window
    pipeline (window = one key chunk of one (pair, query-block)),
    processed in groups of two so same-PE-mode matmuls pipeline:
    QK pair x2 -> EXP x2 -> static filler -> lagged PV x2.
  - PV/ssum trail the exp stream by LAG=6 windows (2 at the kernel tail)
    and cross query-block boundaries (two O^T PSUM accumulators in
    flight), decoupling the in-order PE's PV/projection backlog from the
    QK->EXP critical chain.
  - Startup: weights arrive pair-split and pre-packed in SBUF layout
    (one contiguous 256KB DMA each); x columns stream in 256/512-col
    slabs.  The first q/k projections chase the per-chunk DMAs, so the
    first EXP fires at ~13us instead of ~29us.  DMA issue order is the
    priority order (descriptors of each dma_start spread over all 16
    queues; the phase runs at aggregate HBM bandwidth).
  - kt0 blocks are computed in N=256 column sub-blocks chasing the x
    slabs; remaining projection blocks are spread as small parts over
    windows with PE slack (pair-1 carries the late qt1 blocks only).
  - Softmax denominators: DVE folds the two fp16 parity accumulators,
    then one ones-matmul per head (PSUM partitions {0,32} via
    tile_position) -- half the baseline's reduction matmuls.
  - PSUM budget (8 banks): st double-buffer 4 + two O^T accumulators 2 +
    shared proj/ones pool 2.
Outputs: out_o [2, 128, N] bf16 (pair, head-major O^T rows, queries),
         out_s [2, 2, N]   f32 (pair, head, query sums).
"""

import os
import sys

import ml_dtypes
import numpy as np

for _p in ("/opt/trn_rl_repo",):
    if _p not in sys.path:
        sys.path.insert(0, _p)

import concourse.bass as bass  # noqa: E402
import concourse.tile as tile  # noqa: E402
from concourse import bacc, mybir  # noqa: E402
from concourse.bass_utils import run_bass_kernel_spmd  # noqa: E402

N = 2048  # sequence length
C = 1024  # model dim
D = 64  # head dim
NH = 4  # heads per core
HD = NH * D  # 256 output channels per core
NCORES = 8
KB = N // 128  # 16 key chunks of 128
QB = N // 512  # 4 query blocks of 512
KC = C // 128  # 8 contraction chunks for projections
SCALE = 1.0 / 32.0  # 1 / sqrt(C)
LAG = 6  # PV/ssum windows behind the exp stream

F32 = mybir.dt.float32
BF16 = mybir.dt.bfloat16
FP16 = mybir.dt.float16


def build_kernel(tc, xt, wq_d, wk_d, wv_d, bq, bk, bv, out_o, out_s):
    nc = tc.nc
    Exp = mybir.ActivationFunctionType.Exp

    with (
        tc.tile_pool(name="res", bufs=1) as res,
        tc.tile_pool(name="ppsum", bufs=2, space="PSUM") as ppsum,
        tc.tile_pool(name="stp", bufs=2, space="PSUM") as stp,
        tc.tile_pool(name="opp", bufs=2, space="PSUM") as opp,
        tc.tile_pool(name="ptp", bufs=12) as ptp,
        tc.tile_pool(name="otp", bufs=2) as otp,
        tc.tile_pool(name="ssp", bufs=2) as ssp,
    ):
        # ---- resident SBUF tensors ----
        # weights arrive pre-packed per head-pair: [128, KC, 128]
        wq_p = [res.tile([128, KC, 128], BF16, tag=f"wq{m}", name=f"wq{m}") for m in range(2)]
        wk_p = [res.tile([128, KC, 128], BF16, tag=f"wk{m}", name=f"wk{m}") for m in range(2)]
        wv_all = res.tile([128, KC, HD], BF16, tag="wv", name="wv")
        xt_sb = [res.tile([128, N], BF16, tag=f"xt{k}", name=f"xt{k}") for k in range(KC)]
        wv_sb = [wv_all[:, k, :] for k in range(KC)]
        qt_sb = [res.tile([128, N], BF16, tag=f"qt{m}", name=f"qt{m}") for m in range(2)]
        kt_sb = [res.tile([128, N], BF16, tag=f"kt{m}", name=f"kt{m}") for m in range(2)]
        v_sb = [res.tile([128, NH, D], FP16, tag=f"v{kb}", name=f"v{kb}") for kb in range(KB)]
        bq_sb = [res.tile([128, 1], F32, tag=f"bq{m}", name=f"bq{m}") for m in range(2)]
        bk_sb = [res.tile([128, 1], F32, tag=f"bk{m}", name=f"bk{m}") for m in range(2)]
        bv_sb = res.tile([128, HD], F32, tag="bv", name="bv")
        ones_sb = res.tile([128, 1], FP16, tag="ones", name="ones")
        warm_sb = res.tile([1, 2], F32, tag="warm", name="warm")
        warmmm_sb = res.tile([128, 64], BF16, tag="warmmm", name="warmmm")

        # ---- input DMAs in strict priority order ----
        nc.sync.dma_start(out=wq_p[0][:], in_=wq_d[0])
        nc.sync.dma_start(out=wk_p[0][:], in_=wk_d[0])
        for m in range(2):
            sl = slice(m * 128, (m + 1) * 128)
            nc.sync.dma_start(out=bq_sb[m][:], in_=bq[sl])
            nc.sync.dma_start(out=bk_sb[m][:], in_=bk[sl])
        bv_bcast = bass.AP(tensor=bv.tensor, offset=bv.offset, ap=[[0, 128]] + list(bv.ap))
        nc.sync.dma_start(out=bv_sb[:], in_=bv_bcast)
        for k in range(KC):
            nc.sync.dma_start(out=xt_sb[k][:, 0:512], in_=xt[k * 128 : (k + 1) * 128, 0:512])
        nc.sync.dma_start(out=wv_all[:], in_=wv_d.rearrange("(k p) n -> p k n", p=128))
        for j in range(2, 8):  # x columns 512:2048 in 256-col slabs
            for k in range(KC):
                nc.sync.dma_start(
                    out=xt_sb[k][:, j * 256 : (j + 1) * 256],
                    in_=xt[k * 128 : (k + 1) * 128, j * 256 : (j + 1) * 256],
                )
        nc.sync.dma_start(out=wq_p[1][:], in_=wq_d[1])
        nc.sync.dma_start(out=wk_p[1][:], in_=wk_d[1])

        nc.vector.memset(ones_sb[:], 1.0)
        # warm up the ACT exp table while DMAs land
        nc.vector.memset(warm_sb[:], 0.0)
        nc.scalar.activation(out=warm_sb[:, 0:1], in_=warm_sb[:, 1:2], func=Exp)
        # warm up the PE (HAM un-throttles after ~3.4us of sustained
        # activity) on junk data so the prologue projections run at 2.4GHz
        nc.vector.memset(warmmm_sb[:], 0.5)
        wps = ppsum.tile([64, 64], F32, tag="qkps", name="wps")
        for i in range(32):
            nc.tensor.matmul(
                out=wps[:],
                lhsT=warmmm_sb[:, 0:64],
                rhs=warmmm_sb[:],
                start=(i == 0),
                stop=(i == 31),
            )

        # ---- building blocks ----
        def proj_qk_part(state, which, m, nb, k0, k1):
            """Chunks [k0, k1) of a q/k projection block [128, 512]."""
            w_p = (wq_p if which == "q" else wk_p)[m]
            nsl = slice(nb * 512, (nb + 1) * 512)
            if k0 == 0:
                state["ps"] = ppsum.tile([128, 512], F32, tag="qkps", name="qkps")
            ps = state["ps"]
            for k in range(k0, k1):
                nc.tensor.matmul(
                    out=ps[:],
                    lhsT=w_p[:, k, :],
                    rhs=xt_sb[k][:, nsl],
                    start=(k == 0),
                    stop=(k == KC - 1),
                )
            if k1 == KC:
                b_sb = (bq_sb if which == "q" else bk_sb)[m]
                t_sb = (qt_sb if which == "q" else kt_sb)[m]
                nc.vector.tensor_scalar_add(out=t_sb[:, nsl], in0=ps[:], scalar1=b_sb[:])

        def proj_qk_256(which, m, nb2):
            """One N=256 column sub-block of a q/k projection (chases the
            256-col x slabs)."""
            w_p = (wq_p if which == "q" else wk_p)[m]
            nsl = slice(nb2 * 256, (nb2 + 1) * 256)
            ps = ppsum.tile([128, 256], F32, tag="qkps", name="qkps2")
            for k in range(KC):
                nc.tensor.matmul(
                    out=ps[:],
                    lhsT=w_p[:, k, :],
                    rhs=xt_sb[k][:, nsl],
                    start=(k == 0),
                    stop=(k == KC - 1),
                )
            b_sb = (bq_sb if which == "q" else bk_sb)[m]
            t_sb = (qt_sb if which == "q" else kt_sb)[m]
            nc.vector.tensor_scalar_add(out=t_sb[:, nsl], in0=ps[:], scalar1=b_sb[:])

        def proj_v_block(kb):
            vps = ppsum.tile([128, HD], F32, tag="qkps", name="vps")
            for k in range(KC):
                nc.tensor.matmul(
                    out=vps[:],
                    lhsT=xt_sb[k][:, kb * 128 : (kb + 1) * 128],
                    rhs=wv_sb[k][:],
                    start=(k == 0),
                    stop=(k == KC - 1),
                )
            nc.vector.tensor_add(
                out=v_sb[kb][:],
                in0=vps[:].rearrange("p (h d) -> p h d", h=NH),
                in1=bv_sb[:].rearrange("p (h d) -> p h d", h=NH),
            )

        # ---- static filler schedule: window -> list of closures ----
        filler = {}

        def sched(w, fn):
            filler.setdefault(w, []).append(fn)

        def sched_parts(windows, which, m, nb):
            st = {}
            bounds = [round(i * KC / len(windows)) for i in range(len(windows) + 1)]
            for w, k0, k1 in zip(windows, bounds, bounds[1:]):
                sched(w, lambda st=st, k0=k0, k1=k1: proj_qk_part(st, which, m, nb, k0, k1))

        for kb in range(KB):  # v(kb) three windows ahead of its PV
            sched(kb + 3, lambda kb=kb: proj_v_block(kb))
        # kt0 column sub-blocks chase the x slab DMAs
        for w, nb2 in ((1, 2), (2, 3), (5, 4), (6, 5), (9, 6), (10, 7)):
            sched(w, lambda nb2=nb2: proj_qk_256("k", 0, nb2))
        sched_parts([12, 13], "q", 0, 1)
        sched_parts([20, 21, 22, 23], "q", 0, 2)
        sched_parts([33, 34, 35, 36], "q", 0, 3)
        sched_parts([38, 39, 40, 41], "k", 1, 0)
        sched_parts([43, 44, 45, 46], "k", 1, 1)
        sched_parts([48, 49, 50, 51], "k", 1, 2)
        sched_parts([53, 54, 55, 56], "k", 1, 3)
        sched_parts([58, 59, 60, 61], "q", 1, 0)
        sched_parts([72, 73, 74, 75], "q", 1, 1)
        sched_parts([88, 89, 90, 91], "q", 1, 2)
        sched_parts([104, 105, 106, 107], "q", 1, 3)

        # ---- prologue: first projections chase the per-chunk x DMAs ----
        k0ps = ppsum.tile([128, 512], F32, tag="qkps", name="k0ps")
        q0ps = ppsum.tile([128, 512], F32, tag="qkps", name="q0ps")
        for k in range(KC):
            nc.tensor.matmul(
                out=k0ps[:], lhsT=wk_p[0][:, k, :], rhs=xt_sb[k][:, 0:512],
                start=(k == 0), stop=(k == KC - 1),
            )
            nc.tensor.matmul(
                out=q0ps[:], lhsT=wq_p[0][:, k, :], rhs=xt_sb[k][:, 0:512],
                start=(k == 0), stop=(k == KC - 1),
            )
        nc.vector.tensor_scalar_add(out=kt_sb[0][:, 0:512], in0=k0ps[:], scalar1=bk_sb[0][:])
        nc.vector.tensor_scalar_add(out=qt_sb[0][:, 0:512], in0=q0ps[:], scalar1=bq_sb[0][:])

        # ---- the flat lagged window pipeline ----
        qstate = {}

        def emit_pv(p, qb, kb, pt):
            if (p, qb) not in qstate:
                qstate[(p, qb)] = {
                    "o": opp.tile([128, 512], F32, tag="o", name="o2"),
                    "ssum": [
                        ssp.tile([128, 2, 512], FP16, tag=f"ssum{j}", name=f"ssum{j}")
                        for j in range(2)
                    ],
                }
            s = qstate[(p, qb)]
            o_ps = s["o"]
            for h in range(2):
                nc.tensor.matmul(
                    out=o_ps[h * D : (h + 1) * D, :],
                    lhsT=v_sb[kb][:, 2 * p + h, :],
                    rhs=pt[:, h, :],
                    start=(kb == 0),
                    stop=(kb == KB - 1),
                    tile_position=(0, h * D),
                    skip_group_check=True,
                )
            sj = s["ssum"][kb % 2]
            if kb < 2:
                nc.vector.tensor_copy(out=sj[:], in_=pt[:])
            else:
                nc.vector.tensor_add(out=sj[:], in0=sj[:], in1=pt[:])

        def epilogue(p, qb):
            s = qstate.pop((p, qb))
            qsl = slice(qb * 512, (qb + 1) * 512)
            ssum = s["ssum"]
            nc.vector.tensor_add(out=ssum[0][:], in0=ssum[0][:], in1=ssum[1][:])
            s_ps = ppsum.tile([33, 512], F32, tag="qkps", name="sps")
            for h in range(2):
                nc.tensor.matmul(
                    out=s_ps[32 * h : 32 * h + 1, :],
                    lhsT=ones_sb[:],
                    rhs=ssum[0][:, h, :],
                    start=True,
                    stop=True,
                    tile_position=(0, 32 * h),
                    skip_group_check=True,
                )
            ss = otp.tile([33, 512], F32, tag="ss", name="ss")
            for h in range(2):
                nc.vector.tensor_copy(
                    out=ss[32 * h : 32 * h + 1, :], in_=s_ps[32 * h : 32 * h + 1, :]
                )
            ss_view = bass.AP(
                tensor=ss.tensor, offset=ss.offset,
                ap=[[32 * ss.ap[0][0], 2]] + list(ss.ap[1:]),
            )
            nc.sync.dma_start(out=out_s[p, :, qsl], in_=ss_view)
            ot = otp.tile([128, 512], BF16, tag="ot", name="ot")
            nc.vector.tensor_copy(out=ot[:], in_=s["o"][:])
            nc.sync.dma_start(out=out_o[p, :, qsl], in_=ot[:])

        # Windows are processed in groups of two -- both QK pairs, then
        # both EXPs, then filler, then both lagged PVs -- so same-PE-mode
        # matmuls sit back to back and tiling-mode-switch drains happen
        # once per group instead of once per window.  The 2-slot st pool
        # still pipelines: QK(w+1) fills slot B while exp(w) reads A, and
        # QK(w+2) reuses A which exp(w) freed a full window earlier.
        windows = [(p, qb, kb) for p in range(2) for qb in range(QB) for kb in range(KB)]
        pending = {}

        def emit_qk(w):
            p, qb, kb = windows[w]
            qsl = slice(qb * 512, (qb + 1) * 512)
            ksl = slice(kb * 128, (kb + 1) * 128)
            st = stp.tile([128, 2, 512], F32, tag="st", name="st")
            for h in range(2):
                hsl = slice(h * D, (h + 1) * D)
                nc.tensor.matmul(
                    out=st[:, h, :],
                    lhsT=kt_sb[p][hsl, ksl],
                    rhs=qt_sb[p][hsl, qsl],
                    start=True,
                    stop=True,
                )
            return st

        def emit_exp(w, st):
            p, qb, kb = windows[w]
            pt = ptp.tile([128, 2, 512], FP16, tag="pt", name="pt")
            nc.scalar.activation(out=pt[:], in_=st[:], func=Exp, scale=SCALE)
            pending[w] = (p, qb, kb, pt)

        def drain_pv(wl):
            pl, ql, kl, ptl = pending.pop(wl)
            emit_pv(pl, ql, kl, ptl)
            if kl == KB - 1:
                epilogue(pl, ql)

        # the last few windows trail by only 2 so the post-exp serial tail
        # stays short; PSUM accumulation order commutes and each qb has its
        # own chain, so the reordering at the boundary is safe
        drains = {}
        for w in range(len(windows)):
            dw = w + (2 if w >= 120 else LAG)
            drains.setdefault(dw, []).append(w)
        for w in range(0, len(windows) + 10, 2):
            if w < len(windows):
                st_a = emit_qk(w)
                st_b = emit_qk(w + 1)
                emit_exp(w, st_a)
                emit_exp(w + 1, st_b)
                for fn in filler.get(w, ()):
                    fn()
                for fn in filler.get(w + 1, ()):
                    fn()
            for dw in (w, w + 1):
                for wl in drains.get(dw, ()):
                    drain_pv(wl)


def build_nc():
    nc = bacc.Bacc(
        "TRN2",
        target_bir_lowering=False,
        debug=False,
        num_devices=NCORES,
        enable_partition_id=False,
    )
    xt = nc.dram_tensor("xt", [C, N], BF16, kind="ExternalInput").ap()
    wq_d = [nc.dram_tensor(f"wq{m}", [128, KC, 128], BF16, kind="ExternalInput").ap() for m in range(2)]
    wk_d = [nc.dram_tensor(f"wk{m}", [128, KC, 128], BF16, kind="ExternalInput").ap() for m in range(2)]
    wv_d = nc.dram_tensor("wvt", [C, HD], BF16, kind="ExternalInput").ap()
    bq = nc.dram_tensor("bq", [HD], F32, kind="ExternalInput").ap()
    bk = nc.dram_tensor("bk", [HD], F32, kind="ExternalInput").ap()
    bv = nc.dram_tensor("bv", [HD], F32, kind="ExternalInput").ap()
    out_o = nc.dram_tensor("out_o", [2, 128, N], BF16, kind="ExternalOutput").ap()
    out_s = nc.dram_tensor("out_s", [2, 2, N], F32, kind="ExternalOutput").ap()

    with tile.TileContext(nc) as tc:
        build_kernel(tc, xt, wq_d, wk_d, wv_d, bq, bk, bv, out_o, out_s)
    nc.compile()
    return nc


def _pack_w(w, m):
    """[C, HD] transposed weight -> pair-m packed [128, KC, 128] bf16."""
    wt = np.asarray(w, np.float32)[:, m * 128 : (m + 1) * 128]  # [C, 128]
    return np.ascontiguousarray(
        wt.reshape(KC, 128, 128).transpose(1, 0, 2)
    ).astype(ml_dtypes.bfloat16)


def shard_inputs(inputs):
    x = np.asarray(inputs["x"], np.float32)
    in_maps = []
    for core in range(NCORES):
        b, g = core // 4, core % 4
        sl = slice(g * HD, (g + 1) * HD)
        wqt = np.asarray(inputs["Wq"], np.float32)[sl, :].T  # [C, HD]
        wkt = np.asarray(inputs["Wk"], np.float32)[sl, :].T
        wvt = np.asarray(inputs["Wv"], np.float32)[sl, :].T
        in_maps.append(
            {
                "xt": np.ascontiguousarray(x[b].T).astype(ml_dtypes.bfloat16),
                "wq0": _pack_w(wqt, 0),
                "wq1": _pack_w(wqt, 1),
                "wk0": _pack_w(wkt, 0),
                "wk1": _pack_w(wkt, 1),
                "wvt": np.ascontiguousarray(wvt).astype(ml_dtypes.bfloat16),
                "bq": np.ascontiguousarray(np.asarray(inputs["bq"], np.float32)[sl]),
                "bk": np.ascontiguousarray(np.asarray(inputs["bk"], np.float32)[sl]),
                "bv": np.ascontiguousarray(np.asarray(inputs["bv"], np.float32)[sl]),
            }
        )
    return in_maps


def assemble(results, B=2):
    out = np.zeros((B, N, C), np.float32)
    for core in range(NCORES):
        b, g = core // 4, core % 4
        oo = np.asarray(results[core]["out_o"], np.float32)  # [2, 128, N]
        os_ = np.asarray(results[core]["out_s"], np.float32)  # [2, 2, N]
        o = oo.reshape(2, 2, D, N)  # [pair, head, d, n]
        on = o / os_[:, :, None, :]
        # [pair, head, d, n] -> [n, pair*2*D + head*D + d]
        out[b, :, g * HD : (g + 1) * HD] = (
            on.transpose(3, 0, 1, 2).reshape(N, HD)
        )
    return out


_NC_CACHE = None


def _get_nc():
    global _NC_CACHE
    if _NC_CACHE is None:
        _NC_CACHE = build_nc()
    return _NC_CACHE


def kernel(**inputs):
    nc = _get_nc()
    in_maps = shard_inputs(inputs)
    res = run_bass_kernel_spmd(
        nc,
        in_maps,
        core_ids=list(range(NCORES)),
        trace=bool(int(os.environ.get("KERNEL_TRACE", "0"))),
    )
    return assemble(res.results, B=int(np.asarray(inputs["x"]).shape[0]))


# revision 37
# speedup vs baseline: 1.0526x; 1.0066x over previous
"""Multi-head attention forward kernel for Trainium2 (8 NeuronCores).

Problem: B=2, N=2048, C=1024, H=16 heads, head_dim=64.
    q = x @ Wq.T + bq  (same for k, v)
    out = softmax(q k^T / sqrt(C)) v       (per head), re-merged to [B, N, C]

Sharding: core = (batch b, head-group g): b = core // 4, g = core % 4.
Each core computes 4 heads of one batch element. No collectives needed --
outputs are disjoint; host gathers and finishes with a cheap epilogue
(normalize by the row-sums and transpose).

Design (trace-driven evolution of the 188us baseline):
  - The EXP stream on ACT (128 ops x ~1.05us, 1 elem/cycle/lane, the only
    engine with exp) is the hard floor; the kernel is one flat 128-window
    pipeline (window = one key chunk of one (pair, query-block)),
    processed in groups of two so same-PE-mode matmuls pipeline:
    QK pair x2 -> EXP x2 -> static filler -> lagged PV x2.
  - PV/ssum trail the exp stream by LAG=10 windows (2 at the kernel tail)
    and cross query-block boundaries (two O^T PSUM accumulators in
    flight), decoupling the in-order PE's PV/projection backlog from the
    QK->EXP critical chain.
  - Startup: weights arrive pair-split and pre-packed in SBUF layout
    (one contiguous 256KB DMA each); x columns stream in 256/512-col
    slabs.  The first q/k projections chase the per-chunk DMAs, so the
    first EXP fires at ~13us instead of ~29us.  DMA issue order is the
    priority order (descriptors of each dma_start spread over all 16
    queues; the phase runs at aggregate HBM bandwidth).
  - kt0 blocks are computed in N=256 column sub-blocks chasing the x
    slabs; remaining projection blocks are spread as small parts over
    windows with PE slack (pair-1 carries the late qt1 blocks only).
  - Softmax denominators: DVE folds the two fp16 parity accumulators,
    then one ones-matmul per head (PSUM partitions {0,32} via
    tile_position) -- half the baseline's reduction matmuls.
  - PSUM budget (8 banks): st double-buffer 4 + two O^T accumulators 2 +
    shared proj/ones pool 2.
Outputs: out_o [2, 128, N] bf16 (pair, head-major O^T rows, queries),
         out_s [2, 2, N]   f32 (pair, head, query sums).
"""

import os
import sys

import ml_dtypes
import numpy as np

for _p in ("/opt/trn_rl_repo",):
    if _p not in sys.path:
        sys.path.insert(0, _p)

import concourse.bass as bass  # noqa: E402
import concourse.tile as tile  # noqa: E402
from concourse import bacc, mybir  # noqa: E402
from concourse.bass_utils import run_bass_kernel_spmd  # noqa: E402

N = 2048  # sequence length
C = 1024  # model dim
D = 64  # head dim
NH = 4  # heads per core
HD = NH * D  # 256 output channels per core
NCORES = 8
KB = N // 128  # 16 key chunks of 128
QB = N // 512  # 4 query blocks of 512
KC = C // 128  # 8 contraction chunks for projections
SCALE = 1.0 / 32.0  # 1 / sqrt(C)
LAG = 10  # PV/ssum windows behind the exp stream

F32 = mybir.dt.float32
BF16 = mybir.dt.bfloat16
FP16 = mybir.dt.float16


def build_kernel(tc, xt, wq_d, wk_d, wv_d, bq, bk, bv, out_o, out_s):
    nc = tc.nc
    Exp = mybir.ActivationFunctionType.Exp

    with (
        tc.tile_pool(name="res", bufs=1) as res,
        tc.tile_pool(name="ppsum", bufs=2, space="PSUM") as ppsum,
        tc.tile_pool(name="stp", bufs=2, space="PSUM") as stp,
        tc.tile_pool(name="opp", bufs=2, space="PSUM") as opp,
        tc.tile_pool(name="ptp", bufs=14) as ptp,
        tc.tile_pool(name="otp", bufs=2) as otp,
        tc.tile_pool(name="ssp", bufs=2) as ssp,
    ):
        # ---- resident SBUF tensors ----
        # weights arrive pre-packed per head-pair: [128, KC, 128]
        wq_p = [res.tile([128, KC, 128], BF16, tag=f"wq{m}", name=f"wq{m}") for m in range(2)]
        wk_p = [res.tile([128, KC, 128], BF16, tag=f"wk{m}", name=f"wk{m}") for m in range(2)]
        wv_all = res.tile([128, KC, HD], BF16, tag="wv", name="wv")
        xt_sb = [res.tile([128, N], BF16, tag=f"xt{k}", name=f"xt{k}") for k in range(KC)]
        wv_sb = [wv_all[:, k, :] for k in range(KC)]
        qt_sb = [res.tile([128, N], BF16, tag=f"qt{m}", name=f"qt{m}") for m in range(2)]
        kt_sb = [res.tile([128, N], BF16, tag=f"kt{m}", name=f"kt{m}") for m in range(2)]
        v_sb = [res.tile([128, NH, D], FP16, tag=f"v{kb}", name=f"v{kb}") for kb in range(KB)]
        bq_sb = [res.tile([128, 1], F32, tag=f"bq{m}", name=f"bq{m}") for m in range(2)]
        bk_sb = [res.tile([128, 1], F32, tag=f"bk{m}", name=f"bk{m}") for m in range(2)]
        bv_sb = res.tile([128, HD], F32, tag="bv", name="bv")
        ones_sb = res.tile([128, 1], FP16, tag="ones", name="ones")
        warm_sb = res.tile([1, 2], F32, tag="warm", name="warm")
        warmmm_sb = res.tile([128, 64], BF16, tag="warmmm", name="warmmm")

        # ---- input DMAs in strict priority order ----
        nc.sync.dma_start(out=wq_p[0][:], in_=wq_d[0])
        nc.sync.dma_start(out=wk_p[0][:], in_=wk_d[0])
        for m in range(2):
            sl = slice(m * 128, (m + 1) * 128)
            nc.sync.dma_start(out=bq_sb[m][:], in_=bq[sl])
            nc.sync.dma_start(out=bk_sb[m][:], in_=bk[sl])
        bv_bcast = bass.AP(tensor=bv.tensor, offset=bv.offset, ap=[[0, 128]] + list(bv.ap))
        nc.sync.dma_start(out=bv_sb[:], in_=bv_bcast)
        for k in range(KC):
            nc.sync.dma_start(out=xt_sb[k][:, 0:512], in_=xt[k * 128 : (k + 1) * 128, 0:512])
        nc.sync.dma_start(out=wv_all[:], in_=wv_d.rearrange("(k p) n -> p k n", p=128))
        for j in range(2, 8):  # x columns 512:2048 in 256-col slabs
            for k in range(KC):
                nc.sync.dma_start(
                    out=xt_sb[k][:, j * 256 : (j + 1) * 256],
                    in_=xt[k * 128 : (k + 1) * 128, j * 256 : (j + 1) * 256],
                )
        nc.sync.dma_start(out=wq_p[1][:], in_=wq_d[1])
        nc.sync.dma_start(out=wk_p[1][:], in_=wk_d[1])

        nc.vector.memset(ones_sb[:], 1.0)
        # warm up the ACT exp table while DMAs land
        nc.vector.memset(warm_sb[:], 0.0)
        nc.scalar.activation(out=warm_sb[:, 0:1], in_=warm_sb[:, 1:2], func=Exp)
        # warm up the PE (HAM un-throttles after ~3.4us of sustained
        # activity) on junk data so the prologue projections run at 2.4GHz
        nc.vector.memset(warmmm_sb[:], 0.5)
        wps = ppsum.tile([64, 64], F32, tag="qkps", name="wps")
        for i in range(32):
            nc.tensor.matmul(
                out=wps[:],
                lhsT=warmmm_sb[:, 0:64],
                rhs=warmmm_sb[:],
                start=(i == 0),
                stop=(i == 31),
            )

        # ---- building blocks ----
        def proj_qk_part(state, which, m, nb, k0, k1):
            """Chunks [k0, k1) of a q/k projection block [128, 512]."""
            w_p = (wq_p if which == "q" else wk_p)[m]
            nsl = slice(nb * 512, (nb + 1) * 512)
            if k0 == 0:
                state["ps"] = ppsum.tile([128, 512], F32, tag="qkps", name="qkps")
            ps = state["ps"]
            for k in range(k0, k1):
                nc.tensor.matmul(
                    out=ps[:],
                    lhsT=w_p[:, k, :],
                    rhs=xt_sb[k][:, nsl],
                    start=(k == 0),
                    stop=(k == KC - 1),
                )
            if k1 == KC:
                b_sb = (bq_sb if which == "q" else bk_sb)[m]
                t_sb = (qt_sb if which == "q" else kt_sb)[m]
                nc.vector.tensor_scalar_add(out=t_sb[:, nsl], in0=ps[:], scalar1=b_sb[:])

        def proj_qk_256(which, m, nb2):
            """One N=256 column sub-block of a q/k projection (chases the
            256-col x slabs)."""
            w_p = (wq_p if which == "q" else wk_p)[m]
            nsl = slice(nb2 * 256, (nb2 + 1) * 256)
            ps = ppsum.tile([128, 256], F32, tag="qkps", name="qkps2")
            for k in range(KC):
                nc.tensor.matmul(
                    out=ps[:],
                    lhsT=w_p[:, k, :],
                    rhs=xt_sb[k][:, nsl],
                    start=(k == 0),
                    stop=(k == KC - 1),
                )
            b_sb = (bq_sb if which == "q" else bk_sb)[m]
            t_sb = (qt_sb if which == "q" else kt_sb)[m]
            nc.vector.tensor_scalar_add(out=t_sb[:, nsl], in0=ps[:], scalar1=b_sb[:])

        def proj_v_block(kb):
            vps = ppsum.tile([128, HD], F32, tag="qkps", name="vps")
            for k in range(KC):
                nc.tensor.matmul(
                    out=vps[:],
                    lhsT=xt_sb[k][:, kb * 128 : (kb + 1) * 128],
                    rhs=wv_sb[k][:],
                    start=(k == 0),
                    stop=(k == KC - 1),
                )
            nc.vector.tensor_add(
                out=v_sb[kb][:],
                in0=vps[:].rearrange("p (h d) -> p h d", h=NH),
                in1=bv_sb[:].rearrange("p (h d) -> p h d", h=NH),
            )

        # ---- static filler schedule: window -> list of closures ----
        filler = {}

        def sched(w, fn):
            filler.setdefault(w, []).append(fn)

        def sched_parts(windows, which, m, nb):
            st = {}
            bounds = [round(i * KC / len(windows)) for i in range(len(windows) + 1)]
            for w, k0, k1 in zip(windows, bounds, bounds[1:]):
                sched(w, lambda st=st, k0=k0, k1=k1: proj_qk_part(st, which, m, nb, k0, k1))

        for kb in range(KB):  # v(kb) three windows ahead of its PV
            sched(kb + 3, lambda kb=kb: proj_v_block(kb))
        # kt0 column sub-blocks chase the x slab DMAs
        for w, nb2 in ((1, 2), (2, 3), (5, 4), (6, 5), (9, 6), (10, 7)):
            sched(w, lambda nb2=nb2: proj_qk_256("k", 0, nb2))
        sched_parts([12, 13], "q", 0, 1)
        sched_parts([20, 21, 22, 23], "q", 0, 2)
        sched_parts([33, 34, 35, 36], "q", 0, 3)
        sched_parts([38, 39, 40, 41], "k", 1, 0)
        sched_parts([43, 44, 45, 46], "k", 1, 1)
        sched_parts([48, 49, 50, 51], "k", 1, 2)
        sched_parts([53, 54, 55, 56], "k", 1, 3)
        sched_parts([58, 59, 60, 61], "q", 1, 0)
        sched_parts([72, 73, 74, 75], "q", 1, 1)
        sched_parts([88, 89, 90, 91], "q", 1, 2)
        sched_parts([104, 105, 106, 107], "q", 1, 3)

        # ---- prologue: first projections chase the per-chunk x DMAs ----
        k0ps = ppsum.tile([128, 512], F32, tag="qkps", name="k0ps")
        q0ps = ppsum.tile([128, 512], F32, tag="qkps", name="q0ps")
        for k in range(KC):
            nc.tensor.matmul(
                out=k0ps[:], lhsT=wk_p[0][:, k, :], rhs=xt_sb[k][:, 0:512],
                start=(k == 0), stop=(k == KC - 1),
            )
            nc.tensor.matmul(
                out=q0ps[:], lhsT=wq_p[0][:, k, :], rhs=xt_sb[k][:, 0:512],
                start=(k == 0), stop=(k == KC - 1),
            )
        nc.vector.tensor_scalar_add(out=kt_sb[0][:, 0:512], in0=k0ps[:], scalar1=bk_sb[0][:])
        nc.vector.tensor_scalar_add(out=qt_sb[0][:, 0:512], in0=q0ps[:], scalar1=bq_sb[0][:])

        # ---- the flat lagged window pipeline ----
        qstate = {}

        def emit_pv(p, qb, kb, pt):
            if (p, qb) not in qstate:
                qstate[(p, qb)] = {
                    "o": opp.tile([128, 512], F32, tag="o", name="o2"),
                    "ssum": [
                        ssp.tile([128, 2, 512], FP16, tag=f"ssum{j}", name=f"ssum{j}")
                        for j in range(2)
                    ],
                }
            s = qstate[(p, qb)]
            o_ps = s["o"]
            for h in range(2):
                nc.tensor.matmul(
                    out=o_ps[h * D : (h + 1) * D, :],
                    lhsT=v_sb[kb][:, 2 * p + h, :],
                    rhs=pt[:, h, :],
                    start=(kb == 0),
                    stop=(kb == KB - 1),
                    tile_position=(0, h * D),
                    skip_group_check=True,
                )
            sj = s["ssum"][kb % 2]
            if kb < 2:
                nc.vector.tensor_copy(out=sj[:], in_=pt[:])
            else:
                nc.vector.tensor_add(out=sj[:], in0=sj[:], in1=pt[:])

        def epilogue(p, qb):
            s = qstate.pop((p, qb))
            qsl = slice(qb * 512, (qb + 1) * 512)
            ssum = s["ssum"]
            nc.vector.tensor_add(out=ssum[0][:], in0=ssum[0][:], in1=ssum[1][:])
            s_ps = ppsum.tile([33, 512], F32, tag="qkps", name="sps")
            for h in range(2):
                nc.tensor.matmul(
                    out=s_ps[32 * h : 32 * h + 1, :],
                    lhsT=ones_sb[:],
                    rhs=ssum[0][:, h, :],
                    start=True,
                    stop=True,
                    tile_position=(0, 32 * h),
                    skip_group_check=True,
                )
            ss = otp.tile([33, 512], F32, tag="ss", name="ss")
            for h in range(2):
                nc.vector.tensor_copy(
                    out=ss[32 * h : 32 * h + 1, :], in_=s_ps[32 * h : 32 * h + 1, :]
                )
            ss_view = bass.AP(
                tensor=ss.tensor, offset=ss.offset,
                ap=[[32 * ss.ap[0][0], 2]] + list(ss.ap[1:]),
            )
            nc.sync.dma_start(out=out_s[p, :, qsl], in_=ss_view)
            ot = otp.tile([128, 512], BF16, tag="ot", name="ot")
            nc.vector.tensor_copy(out=ot[:], in_=s["o"][:])
            nc.sync.dma_start(out=out_o[p, :, qsl], in_=ot[:])

        # Windows are processed in groups of two -- both QK pairs, then
        # both EXPs, then filler, then both lagged PVs -- so same-PE-mode
        # matmuls sit back to back and tiling-mode-switch drains happen
        # once per group instead of once per window.  The 2-slot st pool
        # still pipelines: QK(w+1) fills slot B while exp(w) reads A, and
        # QK(w+2) reuses A which exp(w) freed a full window earlier.
        windows = [(p, qb, kb) for p in range(2) for qb in range(QB) for kb in range(KB)]
        pending = {}

        def emit_qk(w):
            p, qb, kb = windows[w]
            qsl = slice(qb * 512, (qb + 1) * 512)
            ksl = slice(kb * 128, (kb + 1) * 128)
            st = stp.tile([128, 2, 512], F32, tag="st", name="st")
            for h in range(2):
                hsl = slice(h * D, (h + 1) * D)
                nc.tensor.matmul(
                    out=st[:, h, :],
                    lhsT=kt_sb[p][hsl, ksl],
                    rhs=qt_sb[p][hsl, qsl],
                    start=True,
                    stop=True,
                )
            return st

        def emit_exp(w, st):
            p, qb, kb = windows[w]
            pt = ptp.tile([128, 2, 512], FP16, tag="pt", name="pt")
            nc.scalar.activation(out=pt[:], in_=st[:], func=Exp, scale=SCALE)
            pending[w] = (p, qb, kb, pt)

        def drain_pv(wl):
            pl, ql, kl, ptl = pending.pop(wl)
            emit_pv(pl, ql, kl, ptl)
            if kl == KB - 1:
                epilogue(pl, ql)

        # the last few windows trail by only 2 so the post-exp serial tail
        # stays short; PSUM accumulation order commutes and each qb has its
        # own chain, so the reordering at the boundary is safe
        drains = {}
        for w in range(len(windows)):
            dw = w + (2 if w >= 120 else LAG)
            drains.setdefault(dw, []).append(w)
        for w in range(0, len(windows) + 10, 2):
            if w < len(windows):
                st_a = emit_qk(w)
                st_b = emit_qk(w + 1)
                emit_exp(w, st_a)
                emit_exp(w + 1, st_b)
                for fn in filler.get(w, ()):
                    fn()
                for fn in filler.get(w + 1, ()):
                    fn()
            for dw in (w, w + 1):
                for wl in drains.get(dw, ()):
                    drain_pv(wl)


def build_nc():
    nc = bacc.Bacc(
        "TRN2",
        target_bir_lowering=False,
        debug=False,
        num_devices=NCORES,
        enable_partition_id=False,
    )
    xt = nc.dram_tensor("xt", [C, N], BF16, kind="ExternalInput").ap()
    wq_d = [nc.dram_tensor(f"wq{m}", [128, KC, 128], BF16, kind="ExternalInput").ap() for m in range(2)]
    wk_d = [nc.dram_tensor(f"wk{m}", [128, KC, 128], BF16, kind="ExternalInput").ap() for m in range(2)]
    wv_d = nc.dram_tensor("wvt", [C, HD], BF16, kind="ExternalInput").ap()
    bq = nc.dram_tensor("bq", [HD], F32, kind="ExternalInput").ap()
    bk = nc.dram_tensor("bk", [HD], F32, kind="ExternalInput").ap()
    bv = nc.dram_tensor("bv", [HD], F32, kind="ExternalInput").ap()
    out_o = nc.dram_tensor("out_o", [2, 128, N], BF16, kind="ExternalOutput").ap()
    out_s = nc.dram_tensor("out_s", [2, 2, N], F32, kind="ExternalOutput").ap()

    with tile.TileContext(nc) as tc:
        build_kernel(tc, xt, wq_d, wk_d, wv_d, bq, bk, bv, out_o, out_s)
    nc.compile()
    return nc


def _pack_w(w, m):
    """[C, HD] transposed weight -> pair-m packed [128, KC, 128] bf16."""
    wt = np.asarray(w, np.float32)[:, m * 128 : (m + 1) * 128]  # [C, 128]
    return np.ascontiguousarray(
        wt.reshape(KC, 128, 128).transpose(1, 0, 2)
    ).astype(ml_dtypes.bfloat16)


def shard_inputs(inputs):
    x = np.asarray(inputs["x"], np.float32)
    in_maps = []
    for core in range(NCORES):
        b, g = core // 4, core % 4
        sl = slice(g * HD, (g + 1) * HD)
        wqt = np.asarray(inputs["Wq"], np.float32)[sl, :].T  # [C, HD]
        wkt = np.asarray(inputs["Wk"], np.float32)[sl, :].T
        wvt = np.asarray(inputs["Wv"], np.float32)[sl, :].T
        in_maps.append(
            {
                "xt": np.ascontiguousarray(x[b].T).astype(ml_dtypes.bfloat16),
                "wq0": _pack_w(wqt, 0),
                "wq1": _pack_w(wqt, 1),
                "wk0": _pack_w(wkt, 0),
                "wk1": _pack_w(wkt, 1),
                "wvt": np.ascontiguousarray(wvt).astype(ml_dtypes.bfloat16),
                "bq": np.ascontiguousarray(np.asarray(inputs["bq"], np.float32)[sl]),
                "bk": np.ascontiguousarray(np.asarray(inputs["bk"], np.float32)[sl]),
                "bv": np.ascontiguousarray(np.asarray(inputs["bv"], np.float32)[sl]),
            }
        )
    return in_maps


def assemble(results, B=2):
    out = np.zeros((B, N, C), np.float32)
    for core in range(NCORES):
        b, g = core // 4, core % 4
        oo = np.asarray(results[core]["out_o"], np.float32)  # [2, 128, N]
        os_ = np.asarray(results[core]["out_s"], np.float32)  # [2, 2, N]
        o = oo.reshape(2, 2, D, N)  # [pair, head, d, n]
        on = o / os_[:, :, None, :]
        # [pair, head, d, n] -> [n, pair*2*D + head*D + d]
        out[b, :, g * HD : (g + 1) * HD] = (
            on.transpose(3, 0, 1, 2).reshape(N, HD)
        )
    return out


_NC_CACHE = None


def _get_nc():
    global _NC_CACHE
    if _NC_CACHE is None:
        _NC_CACHE = build_nc()
    return _NC_CACHE


def kernel(**inputs):
    nc = _get_nc()
    in_maps = shard_inputs(inputs)
    res = run_bass_kernel_spmd(
        nc,
        in_maps,
        core_ids=list(range(NCORES)),
        trace=bool(int(os.environ.get("KERNEL_TRACE", "0"))),
    )
    return assemble(res.results, B=int(np.asarray(inputs["x"]).shape[0]))
